# revision 15
# baseline (speedup 1.0000x reference)
"""BTT (block tensor-train) structured FC kernel for Trainium2, 8-core data parallel.

Math: y[b, (oa ob oc od)] = sum_blk sum_{r*} F0[ia,oa,ra] F1[ib,ob,rb] F2[ic,oc,rc]
F3[id,od,rd] C[rd,rc,rb,ra] x[b, (ia ib ic id)]  with all mode dims 8, ranks 2.

Two-stage monarch form (per block blk, rank-pair chunks c=(blk,q), q=(rc,rd)):
  stage A: u[b, iaib, c, ocod] = sum_icid x[b, icid, iaib] * G[icid, c, ocod]
  stage B: y[b, oaob, ocod]    = sum_{c, iaib} W[c][iaib, oaob] * u[b, iaib, c, ocod]

v3 layout (per core, 128 batch rows = 64 bps of 2 samples; bpp j = bps 2j,2j+1):
  stage A runs as 2x row-tiled K=64 matmuls (tile_position (0,0) and (64,0))
  that execute concurrently on the PE: bp 2j's x lives on partitions 0:64,
  bp 2j+1's on 64:128; G is duplicated on both halves. M=128=(par,iaib).
  PSUM [128,1024] per bp is drained in one [128,1024] copy (DVE/Act alternate).
  stage B: parity-blockdiag W stationary [K=(par,iaib), M=(par,oaob)], moving
  u[:, 8 bps, c, :], accumulating 16 chunks into one psum bank per group.
  y leaves the chip as bf16.
"""

import os

import numpy as np

N_CORES = 8
B_CORE = 128

_CACHE = {}


def _fold_weights_base(cores, factors):
    cores = np.asarray(cores, dtype=np.float64)      # (4, 2,2,2,2) [rd,rc,rb,ra]
    factors = np.asarray(factors, dtype=np.float64)  # (4, 4, 8, 8, 2)
    G = np.zeros((64, 4, 4, 64), np.float64)         # [icid, blk, q, ocod]
    W = np.zeros((4, 4, 64, 64), np.float64)         # [blk, q, iaib, oaob]
    for blk in range(4):
        F0, F1, F2, F3 = (factors[blk, j] for j in range(4))
        C = cores[blk]
        G[:, blk] = np.einsum("cxr,dys->cdrsxy", F2, F3).reshape(64, 4, 64)
        w = np.einsum("srqp,axp,byq->srabxy", C, F0, F1).transpose(1, 0, 2, 3, 4, 5)
        W[blk] = w.reshape(4, 64, 64)
    g2 = G.reshape(64, 1024)                               # [icid, (blk q ocod)]
    w3 = W.reshape(16, 64, 64)                             # [c, iaib, oaob]
    return g2, w3


def _fold_weights(cores, factors):
    import ml_dtypes
    g2, w3 = _fold_weights_base(cores, factors)
    g_dup = np.concatenate([g2, g2], axis=0)               # [128, 1024]
    w_bd = np.zeros((128, 16, 128), np.float64)            # [(par,iaib), c, (par,oaob)]
    for c in range(16):
        w_bd[0:64, c, 0:64] = w3[c]
        w_bd[64:128, c, 64:128] = w3[c]
    return (g_dup.astype(ml_dtypes.bfloat16),
            np.ascontiguousarray(w_bd.reshape(128, 2048)).astype(ml_dtypes.bfloat16))


def _build_nc():
    import concourse.mybir as mybir
    from concourse import bacc
    from concourse.tile import TileContext

    f32 = mybir.dt.float32
    bf16 = mybir.dt.bfloat16

    nc = bacc.Bacc("TRN2", target_bir_lowering=False, debug=False,
                   num_devices=N_CORES)
    # xt: [p=(half, icid), bpp, (par, iaib)]; bp 2j on partitions 0:64, 2j+1 on 64:128
    xt_d = nc.dram_tensor("xt", [128, 32, 128], bf16, kind="ExternalInput")
    g_d = nc.dram_tensor("g", [128, 1024], bf16, kind="ExternalInput")
    w_d = nc.dram_tensor("w", [128, 2048], bf16, kind="ExternalInput")
    # y: [p=(par, oaob), (group, bpig, ocod)] bf16
    y_d = nc.dram_tensor("y", [128, 4096], bf16, kind="ExternalOutput")

    with TileContext(nc) as tc:
        with tc.tile_pool(name="const", bufs=1) as const, \
             tc.tile_pool(name="upool", bufs=3) as upool, \
             tc.tile_pool(name="ypool", bufs=2) as ypool:

            warm = const.tile([128, 512], bf16, tag="warm")
            g_sb = const.tile([128, 1024], bf16, tag="g_sb")
            w_sb = const.tile([128, 16, 128], bf16, tag="w_sb")
            xs = const.tile([128, 32, 128], bf16, tag="xs")

            # spread input DMAs over the queues so they land concurrently
            nc.sync.dma_start(g_sb[:], g_d[:])
            nc.sync.dma_start(xs[:, 0:12, :], xt_d[:, 0:12, :])
            nc.scalar.dma_start(xs[:, 12:24, :], xt_d[:, 12:24, :])
            nc.scalar.dma_start(w_sb[:], w_d[:].rearrange("p (c m) -> p c m", c=16))
            nc.gpsimd.dma_start(xs[:, 24:32, :], xt_d[:, 24:32, :])
            nc.vector.memset(warm[:], 0.0)

            with tc.tile_pool(name="apsum", bufs=3, space="PSUM") as apsum, \
                 tc.tile_pool(name="bpsum", bufs=2, space="PSUM") as bpsum:
                # HAM warmup while inputs stream in (short N so real MMs
                # aren't blocked long in the in-order PE queue)
                wps = bpsum.tile([128, 512], f32, tag="bps")
                for _ in range(4):
                    nc.tensor.matmul(wps[:, 0:256], warm[:, 0:128],
                                     warm[:, 0:256], start=True, stop=True)

                eng = [nc.vector.tensor_copy, nc.scalar.copy]

                def b_chunk(psB, u_t, c):
                    nc.tensor.matmul(psB[:], w_sb[:, c, :],
                                     u_t[:, 0:8, c * 64:(c + 1) * 64],
                                     start=(c == 0), stop=(c == 15))

                def y_out(psB, gi):
                    y_g = ypool.tile([128, 512], bf16, tag="y_g")
                    eng[gi % 2](y_g[:], psB[:])
                    nc.sync.dma_start(y_d[:, gi * 512:(gi + 1) * 512], y_g[:])

                u_cur = None
                psB7 = None
                pend_y = None        # finished (psB, gi) awaiting drain
                bq = []    # queued B-chunks: (psB, u_tile, c, gi, push_j)
                for j in range(32):          # bpp index
                    gi, slot = divmod(j, 4)  # 8 groups of 4 bpps (8 bps)
                    if slot == 0:
                        u_cur = upool.tile([128, 8, 1024], bf16, tag="u")
                    psA0 = apsum.tile([128, 1024], f32, tag="aps")
                    psA1 = apsum.tile([128, 1024], f32, tag="aps")
                    lhsA = xs[0:64, j, :]
                    lhsB = xs[64:128, j, :]
                    nc.tensor.matmul(psA0[:, 0:512], lhsA, g_sb[0:64, 0:512],
                                     start=True, stop=True)
                    nc.tensor.matmul(psA1[:, 0:512], lhsB, g_sb[64:128, 0:512],
                                     start=True, stop=True)
                    nc.tensor.matmul(psA0[:, 512:1024], lhsA, g_sb[0:64, 512:1024],
                                     start=True, stop=True)
                    nc.tensor.matmul(psA1[:, 512:1024], lhsB,
                                     g_sb[64:128, 512:1024],
                                     start=True, stop=True)
                    # B-dribble: 4 chunks per bpp, starting one bpp after the
                    # source group's last drain was queued
                    if bq and j > bq[0][4] + 1:
                        for _ in range(4 if j < 29 else 6):
                            if not bq or j <= bq[0][4] + 1:
                                break
                            psB_c, u_t, c, g_src, _ = bq.pop(0)
                            b_chunk(psB_c, u_t, c)
                            if c == 15:
                                pend_y = (psB_c, g_src)
                    elif j < 6:
                        # keep the PE dense during group 0 so HAM stays hot
                        for _ in range(4):
                            nc.tensor.matmul(wps[:], warm[:, 0:128], warm[:],
                                             start=True, stop=True)
                    if j >= 30:
                        # final group's left-half chain (bps 0:4 are drained)
                        if psB7 is None:
                            psB7 = bpsum.tile([128, 512], f32, tag="bps")
                        for c in range(8 * (j - 30), 8 * (j - 30) + 8):
                            nc.tensor.matmul(
                                psB7[:, 0:256], w_sb[:, c, :],
                                u_cur[:, 0:4, c * 64:(c + 1) * 64],
                                start=(c == 0), stop=(c == 15))
                    if j == 31:
                        # split drains so the post-loop chain starts sooner
                        eng[0](u_cur[:, 2 * slot, 0:512], psA0[:, 0:512])
                        eng[1](u_cur[:, 2 * slot, 512:1024], psA0[:, 512:1024])
                        eng[0](u_cur[:, 2 * slot + 1, 0:512], psA1[:, 0:512])
                        eng[1](u_cur[:, 2 * slot + 1, 512:1024],
                               psA1[:, 512:1024])
                    else:
                        eng[j % 2](u_cur[:, 2 * slot, :], psA0[:])
                        eng[1 - j % 2](u_cur[:, 2 * slot + 1, :], psA1[:])
                    if pend_y is not None:
                        y_out(*pend_y)
                        pend_y = None

                    if slot == 3 and gi < 7:
                        psB = bpsum.tile([128, 512], f32, tag="bps")
                        bq += [(psB, u_cur, c, gi, j) for c in range(16)]

                # remaining dribble chunks (group 6 tail)
                for psB_c, u_t, c, g_src, _ in bq:
                    b_chunk(psB_c, u_t, c)
                    if c == 15:
                        pend_y = (psB_c, g_src)
                # final group's right-half chain
                for c in range(16):
                    nc.tensor.matmul(
                        psB7[:, 256:512], w_sb[:, c, :],
                        u_cur[:, 4:8, c * 64:(c + 1) * 64],
                        start=(c == 0), stop=(c == 15))
                if pend_y is not None:
                    y_out(*pend_y)
                # drain final y with both engines in halves
                y_g = ypool.tile([128, 512], bf16, tag="y_g")
                eng[0](y_g[:, 0:256], psB7[:, 0:256])
                eng[1](y_g[:, 256:512], psB7[:, 256:512])
                nc.sync.dma_start(y_d[:, 7 * 512:8 * 512], y_g[:])

    nc.compile()
    return nc


def kernel(inputs, cores, factors, trace=False):
    import ml_dtypes
    x = np.ascontiguousarray(np.asarray(inputs, dtype=np.float32))
    assert x.shape == (N_CORES * B_CORE, 4096), x.shape
    g_dup, w_host = _fold_weights(cores, factors)

    from concourse.bass_utils import run_bass_kernel_spmd

    if "nc" not in _CACHE:
        _CACHE["nc"] = _build_nc()
    nc = _CACHE["nc"]

    in_maps = []
    for cidx in range(N_CORES):
        xc = x[cidx * B_CORE:(cidx + 1) * B_CORE].reshape(32, 2, 2, 64, 64)
        # [j, i, par, iaib, icid] -> [(i, icid), j, (par, iaib)]
        xt = np.ascontiguousarray(
            xc.transpose(1, 4, 0, 2, 3).reshape(128, 32, 128)
        ).astype(ml_dtypes.bfloat16)
        in_maps.append({"xt": xt, "g": g_dup, "w": w_host})

    res = run_bass_kernel_spmd(nc, in_maps, core_ids=list(range(N_CORES)),
                               trace=trace)
    _CACHE["last_result"] = res

    out = np.empty((N_CORES * B_CORE, 4096), np.float32)
    for cidx in range(N_CORES):
        yp = np.asarray(res.results[cidx]["y"], dtype=np.float32)  # [128, 4096]
        yr = yp.reshape(2, 64, 8, 8, 64)       # [par, oaob, g, bpig, ocod]
        yb = yr.transpose(2, 3, 0, 1, 4).reshape(128, 4096)
        out[cidx * B_CORE:(cidx + 1) * B_CORE] = yb
    return out


# revision 16
# speedup vs baseline: 1.0365x; 1.0365x over previous
"""BTT (block tensor-train) structured FC kernel for Trainium2, 8-core data parallel.

Math: y[b, (oa ob oc od)] = sum_blk sum_{r*} F0[ia,oa,ra] F1[ib,ob,rb] F2[ic,oc,rc]
F3[id,od,rd] C[rd,rc,rb,ra] x[b, (ia ib ic id)]  with all mode dims 8, ranks 2.

Two-stage monarch form (per block blk, rank-pair chunks c=(blk,q), q=(rc,rd)):
  stage A: u[b, iaib, c, ocod] = sum_icid x[b, icid, iaib] * G[icid, c, ocod]
  stage B: y[b, oaob, ocod]    = sum_{c, iaib} W[c][iaib, oaob] * u[b, iaib, c, ocod]

v3 layout (per core, 128 batch rows = 64 bps of 2 samples; bpp j = bps 2j,2j+1):
  stage A runs as 2x row-tiled K=64 matmuls (tile_position (0,0) and (64,0))
  that execute concurrently on the PE: bp 2j's x lives on partitions 0:64,
  bp 2j+1's on 64:128; G is duplicated on both halves. M=128=(par,iaib).
  PSUM [128,1024] per bp is drained in one [128,1024] copy (DVE/Act alternate).
  stage B: parity-blockdiag W stationary [K=(par,iaib), M=(par,oaob)], moving
  u[:, 8 bps, c, :], accumulating 16 chunks into one psum bank per group.
  y leaves the chip as bf16.
"""

import os

import numpy as np

N_CORES = 8
B_CORE = 128

_CACHE = {}


def _fold_weights_base(cores, factors):
    cores = np.asarray(cores, dtype=np.float64)      # (4, 2,2,2,2) [rd,rc,rb,ra]
    factors = np.asarray(factors, dtype=np.float64)  # (4, 4, 8, 8, 2)
    G = np.zeros((64, 4, 4, 64), np.float64)         # [icid, blk, q, ocod]
    W = np.zeros((4, 4, 64, 64), np.float64)         # [blk, q, iaib, oaob]
    for blk in range(4):
        F0, F1, F2, F3 = (factors[blk, j] for j in range(4))
        C = cores[blk]
        G[:, blk] = np.einsum("cxr,dys->cdrsxy", F2, F3).reshape(64, 4, 64)
        w = np.einsum("srqp,axp,byq->srabxy", C, F0, F1).transpose(1, 0, 2, 3, 4, 5)
        W[blk] = w.reshape(4, 64, 64)
    g2 = G.reshape(64, 1024)                               # [icid, (blk q ocod)]
    w3 = W.reshape(16, 64, 64)                             # [c, iaib, oaob]
    return g2, w3


def _fold_weights(cores, factors):
    import ml_dtypes
    g2, w3 = _fold_weights_base(cores, factors)
    g_dup = np.concatenate([g2, g2], axis=0)               # [128, 1024]
    w_bd = np.zeros((128, 16, 128), np.float64)            # [(par,iaib), c, (par,oaob)]
    for c in range(16):
        w_bd[0:64, c, 0:64] = w3[c]
        w_bd[64:128, c, 64:128] = w3[c]
    return (g_dup.astype(ml_dtypes.bfloat16),
            np.ascontiguousarray(w_bd.reshape(128, 2048)).astype(ml_dtypes.bfloat16))


def _build_nc():
    import concourse.mybir as mybir
    from concourse import bacc
    from concourse.tile import TileContext

    f32 = mybir.dt.float32
    bf16 = mybir.dt.bfloat16

    nc = bacc.Bacc("TRN2", target_bir_lowering=False, debug=False,
                   num_devices=N_CORES)
    # xt: [p=(half, icid), bpp, (par, iaib)]; bp 2j on partitions 0:64, 2j+1 on 64:128
    xt_d = nc.dram_tensor("xt", [128, 32, 128], bf16, kind="ExternalInput")
    g_d = nc.dram_tensor("g", [128, 1024], bf16, kind="ExternalInput")
    w_d = nc.dram_tensor("w", [128, 2048], bf16, kind="ExternalInput")
    # y: [p=(par, oaob), (group, bpig, ocod)] bf16
    y_d = nc.dram_tensor("y", [128, 4096], bf16, kind="ExternalOutput")

    with TileContext(nc) as tc:
        with tc.tile_pool(name="const", bufs=1) as const, \
             tc.tile_pool(name="upool", bufs=3) as upool, \
             tc.tile_pool(name="ypool", bufs=2) as ypool:

            warm = const.tile([128, 512], bf16, tag="warm")
            g_sb = const.tile([128, 1024], bf16, tag="g_sb")
            w_sb = const.tile([128, 16, 128], bf16, tag="w_sb")
            xs = const.tile([128, 32, 128], bf16, tag="xs")

            # spread input DMAs over the queues; tiny first chunk lands first
            nc.sync.dma_start(xs[:, 0:4, :], xt_d[:, 0:4, :])
            nc.sync.dma_start(g_sb[:], g_d[:])
            nc.sync.dma_start(xs[:, 4:16, :], xt_d[:, 4:16, :])
            nc.scalar.dma_start(xs[:, 16:28, :], xt_d[:, 16:28, :])
            nc.scalar.dma_start(w_sb[:], w_d[:].rearrange("p (c m) -> p c m", c=16))
            nc.gpsimd.dma_start(xs[:, 28:32, :], xt_d[:, 28:32, :])
            nc.vector.memset(warm[:], 0.0)

            with tc.tile_pool(name="apsum", bufs=3, space="PSUM") as apsum, \
                 tc.tile_pool(name="bpsum", bufs=2, space="PSUM") as bpsum:
                # HAM warmup while inputs stream in (short N so real MMs
                # aren't blocked long in the in-order PE queue)
                wps = bpsum.tile([128, 512], f32, tag="bps")
                for _ in range(12):
                    nc.tensor.matmul(wps[:, 0:256], warm[:, 0:128],
                                     warm[:, 0:256], start=True, stop=True)

                eng = [nc.vector.tensor_copy, nc.scalar.copy]

                def b_chunk(psB, u_t, c):
                    nc.tensor.matmul(psB[:], w_sb[:, c, :],
                                     u_t[:, 0:8, c * 64:(c + 1) * 64],
                                     start=(c == 0), stop=(c == 15))

                def y_out(psB, gi):
                    y_g = ypool.tile([128, 512], bf16, tag="y_g")
                    eng[gi % 2](y_g[:], psB[:])
                    nc.sync.dma_start(y_d[:, gi * 512:(gi + 1) * 512], y_g[:])

                u_cur = None
                psB7 = None
                pend_y = None        # finished (psB, gi) awaiting drain
                bq = []    # queued B-chunks: (psB, u_tile, c, gi, push_j)
                for j in range(32):          # bpp index
                    gi, slot = divmod(j, 4)  # 8 groups of 4 bpps (8 bps)
                    if slot == 0:
                        u_cur = upool.tile([128, 8, 1024], bf16, tag="u")
                    psA0 = apsum.tile([128, 1024], f32, tag="aps")
                    psA1 = apsum.tile([128, 1024], f32, tag="aps")
                    lhsA = xs[0:64, j, :]
                    lhsB = xs[64:128, j, :]
                    nc.tensor.matmul(psA0[:, 0:512], lhsA, g_sb[0:64, 0:512],
                                     start=True, stop=True)
                    nc.tensor.matmul(psA1[:, 0:512], lhsB, g_sb[64:128, 0:512],
                                     start=True, stop=True)
                    nc.tensor.matmul(psA0[:, 512:1024], lhsA, g_sb[0:64, 512:1024],
                                     start=True, stop=True)
                    nc.tensor.matmul(psA1[:, 512:1024], lhsB,
                                     g_sb[64:128, 512:1024],
                                     start=True, stop=True)
                    # B-dribble: 4 chunks per bpp, starting one bpp after the
                    # source group's last drain was queued
                    if bq and j > bq[0][4] + 1:
                        for _ in range(4 if j < 29 else 6):
                            if not bq or j <= bq[0][4] + 1:
                                break
                            psB_c, u_t, c, g_src, _ = bq.pop(0)
                            b_chunk(psB_c, u_t, c)
                            if c == 15:
                                pend_y = (psB_c, g_src)
                    elif j < 6:
                        # keep the PE dense during group 0 so HAM stays hot
                        for _ in range(4):
                            nc.tensor.matmul(wps[:], warm[:, 0:128], warm[:],
                                             start=True, stop=True)
                    if j >= 30:
                        # final group's left-half chain (bps 0:4 are drained)
                        if psB7 is None:
                            psB7 = bpsum.tile([128, 512], f32, tag="bps")
                        for c in range(8 * (j - 30), 8 * (j - 30) + 8):
                            nc.tensor.matmul(
                                psB7[:, 0:256], w_sb[:, c, :],
                                u_cur[:, 0:4, c * 64:(c + 1) * 64],
                                start=(c == 0), stop=(c == 15))
                    if j == 31:
                        # split drains so the post-loop chain starts sooner
                        eng[0](u_cur[:, 2 * slot, 0:512], psA0[:, 0:512])
                        eng[1](u_cur[:, 2 * slot, 512:1024], psA0[:, 512:1024])
                        eng[0](u_cur[:, 2 * slot + 1, 0:512], psA1[:, 0:512])
                        eng[1](u_cur[:, 2 * slot + 1, 512:1024],
                               psA1[:, 512:1024])
                    else:
                        eng[j % 2](u_cur[:, 2 * slot, :], psA0[:])
                        eng[1 - j % 2](u_cur[:, 2 * slot + 1, :], psA1[:])
                    if pend_y is not None:
                        y_out(*pend_y)
                        pend_y = None

                    if slot == 3 and gi < 7:
                        psB = bpsum.tile([128, 512], f32, tag="bps")
                        bq += [(psB, u_cur, c, gi, j) for c in range(16)]

                # remaining dribble chunks (group 6 tail)
                for psB_c, u_t, c, g_src, _ in bq:
                    b_chunk(psB_c, u_t, c)
                    if c == 15:
                        pend_y = (psB_c, g_src)
                # final group's right-half chain
                for c in range(16):
                    nc.tensor.matmul(
                        psB7[:, 256:512], w_sb[:, c, :],
                        u_cur[:, 4:8, c * 64:(c + 1) * 64],
                        start=(c == 0), stop=(c == 15))
                if pend_y is not None:
                    y_out(*pend_y)
                # drain final y with both engines in halves
                y_g = ypool.tile([128, 512], bf16, tag="y_g")
                eng[0](y_g[:, 0:256], psB7[:, 0:256])
                eng[1](y_g[:, 256:512], psB7[:, 256:512])
                nc.sync.dma_start(y_d[:, 7 * 512:8 * 512], y_g[:])

    nc.compile()
    return nc


def kernel(inputs, cores, factors, trace=False):
    import ml_dtypes
    x = np.ascontiguousarray(np.asarray(inputs, dtype=np.float32))
    assert x.shape == (N_CORES * B_CORE, 4096), x.shape
    g_dup, w_host = _fold_weights(cores, factors)

    from concourse.bass_utils import run_bass_kernel_spmd

    if "nc" not in _CACHE:
        _CACHE["nc"] = _build_nc()
    nc = _CACHE["nc"]

    in_maps = []
    for cidx in range(N_CORES):
        xc = x[cidx * B_CORE:(cidx + 1) * B_CORE].reshape(32, 2, 2, 64, 64)
        # [j, i, par, iaib, icid] -> [(i, icid), j, (par, iaib)]
        xt = np.ascontiguousarray(
            xc.transpose(1, 4, 0, 2, 3).reshape(128, 32, 128)
        ).astype(ml_dtypes.bfloat16)
        in_maps.append({"xt": xt, "g": g_dup, "w": w_host})

    res = run_bass_kernel_spmd(nc, in_maps, core_ids=list(range(N_CORES)),
                               trace=trace)
    _CACHE["last_result"] = res

    out = np.empty((N_CORES * B_CORE, 4096), np.float32)
    for cidx in range(N_CORES):
        yp = np.asarray(res.results[cidx]["y"], dtype=np.float32)  # [128, 4096]
        yr = yp.reshape(2, 64, 8, 8, 64)       # [par, oaob, g, bpig, ocod]
        yb = yr.transpose(2, 3, 0, 1, 4).reshape(128, 4096)
        out[cidx * B_CORE:(cidx + 1) * B_CORE] = yb
    return out
